# revision 2
# baseline (speedup 1.0000x reference)
"""Trainium2 Bass kernel for nn_Loss_343597383760.

Loss:
    scores = predicted_values[rel_idx, e1_idx, e2_idx]        # [N] gather
    sig    = sigmoid(scores)
    total  = sum(lab*sig + (1-lab)*(1-sig)) = neg + sum(w*sig),  w = 2*lab-1
    loss   = -total / ((1+neg)*N)

Sharding (expert-style, per relation): core c owns relations {2c, 2c+1} of
predicted_values ([2,4096,4096] f32 = 128 MiB per core). Host buckets the
262144 triplets by owning core and converts each to a flat element index into
the local shard.

v2 layout: within a core's bucket the indices are partitioned by label sign
and packed column-major into a [128, 264] int32 plane: positives fill columns
[0,132), negatives fill [132,264); pad slots hold index TOTAL, which points at
an appended 0.0 element of the pv shard so a pad contributes sigmoid(0)=0.5
exactly. No weight tensor is shipped: the ACT engine evaluates sigmoid(+x)
for positive chunks and sigmoid(-x) (= 1-sigmoid(x)) for negative chunks via
the activation scale, and the host undoes the identity exactly:
    sum w*sig(s) = dev_sum - n_neg_dev - 0.5*pads.

Device pipeline per chunk (4 chunks, one SWDGE queue each, sized small-big-
big-small so the first gather starts early and the last drains quickly):
    HWDGE load of that chunk's idx columns (sync engine)
    indirect DMA gather (4B/elem, SWDGE)      g = pv[idx]
    ACT sigmoid(scale=+/-1) with accum_out    out[:,k] = sum_row sigmoid(+-g)
"""

import numpy as np

import concourse.bass as bass
import concourse.bacc as bacc
import concourse.tile as tile
from concourse import mybir
from concourse.bass_utils import run_bass_kernel_spmd

R, E, N = 16, 4096, 262144
NCORES = 8
RPC = R // NCORES            # relations per core
TOTAL = RPC * E * E          # elements in one core's shard
P = 128                      # SBUF partitions
COLS = 264                   # capacity per core = 128*264 = 33792
CAP = P * COLS
POS_COLS = 132               # columns [0,POS_COLS) positive, rest negative
RCAP = P * POS_COLS          # per-sign region capacity (16896)
# (start_col, end_col, act_scale); small first chunk for an early pipeline
# start, small last chunk for a short drain tail.
CHUNKS = [(0, 24, 1.0), (24, 132, 1.0), (132, 240, -1.0), (240, 264, -1.0)]
NQ = 4                       # SWDGE queues; one per chunk

# Set by test harness to capture a neuron-profile trace.
TRACE = False
LAST_RESULTS = None

_NC = None


def _indirect_gather_q(nc, out, in_, in_offset, queue_name):
    """indirect_dma_start with an explicit SWDGE queue (the stock API pins
    qPoolDynamic; distinct queues let ring drains overlap desc-gen)."""
    orig = mybir.InstDMACopy

    def patched(**kw):
        kw["queue"] = queue_name
        return orig(**kw)

    mybir.InstDMACopy = patched
    try:
        return nc.gpsimd.indirect_dma_start(
            out=out, out_offset=None, in_=in_, in_offset=in_offset
        )
    finally:
        mybir.InstDMACopy = orig


def _build_nc():
    f32 = mybir.dt.float32
    i32 = mybir.dt.int32
    nc = bacc.Bacc(num_swdge_queues=NQ)
    pv = nc.declare_dram_parameter("pv", [TOTAL + 1, 1], f32, isOutput=False)
    idxs = nc.declare_dram_parameter("idx", [P, COLS], i32, isOutput=False)
    out = nc.declare_dram_parameter("out", [P, len(CHUNKS)], f32, isOutput=True)

    with (
        tile.TileContext(nc) as tc,
        tc.tile_pool(name="io", bufs=1) as io_pool,
        tc.tile_pool(name="work", bufs=4) as work_pool,
        tc.tile_pool(name="res", bufs=1) as res_pool,
    ):
        outbuf = res_pool.tile([P, len(CHUNKS)], f32)
        its = []
        for k, (a, b, sgn) in enumerate(CHUNKS):
            it = io_pool.tile([P, b - a], i32, tag=f"idx{k}")
            nc.sync.dma_start(out=it[:], in_=idxs[:, a:b])
            its.append(it)
        for k, (a, b, sgn) in enumerate(CHUNKS):
            w = b - a
            g = work_pool.tile([P, w], f32, tag=f"gath{k}")
            _indirect_gather_q(
                nc,
                out=g[:],
                in_=pv[:],
                in_offset=bass.IndirectOffsetOnAxis(ap=its[k][:], axis=0),
                queue_name=f"qPoolDynamic{k % NQ or ''}",
            )
            sg = work_pool.tile([P, w], f32, tag=f"sig{k}")
            nc.scalar.activation(
                out=sg[:],
                in_=g[:],
                func=mybir.ActivationFunctionType.Sigmoid,
                scale=sgn,
                accum_out=outbuf[:, k : k + 1],
            )
        nc.sync.dma_start(out=out[:], in_=outbuf[:])
    nc.finalize()
    return nc


def kernel(predicted_values, rel_idx, e1_idx, e2_idx, labels):
    global _NC, LAST_RESULTS
    pv = np.ascontiguousarray(np.asarray(predicted_values, dtype=np.float32))
    rel = np.asarray(rel_idx, dtype=np.int64)
    e1 = np.asarray(e1_idx, dtype=np.int64)
    e2 = np.asarray(e2_idx, dtype=np.int64)
    lab = np.asarray(labels, dtype=np.int64)

    owner = rel // RPC
    local_flat = (rel % RPC) * (E * E) + e1 * E + e2  # < TOTAL, fits int32
    pos_mask = lab == 1

    pv_flat = pv.reshape(R * E * E)
    host_extra = 0.0   # sum of w*sig for overflow triplets (host-computed)
    correction = 0.0   # sum over cores of (n_neg_dev_c + 0.5*pad_c)
    in_maps = []
    for c in range(NCORES):
        m = owner == c
        fpos = local_flat[m & pos_mask]
        fneg = local_flat[m & ~pos_mask]
        # Host fallback for any sign bucket exceeding its region capacity:
        # compute w*sigmoid(score) for the overflow triplets exactly.
        for fi, sgn in ((fpos, 1.0), (fneg, -1.0)):
            if fi.size > RCAP:
                of = fi[RCAP:] + c * TOTAL
                s = pv_flat[of].astype(np.float64)
                host_extra += sgn * float(np.sum(1.0 / (1.0 + np.exp(-s))))
        fpos = fpos[:RCAP]
        fneg = fneg[:RCAP]
        correction += fneg.size + 0.5 * (CAP - fpos.size - fneg.size)

        idx_lin = np.full(CAP, TOTAL, np.int32)  # pads gather the 0.0 slot
        idx_lin[: fpos.size] = fpos.astype(np.int32)
        idx_lin[RCAP : RCAP + fneg.size] = fneg.astype(np.int32)
        # slot (p, col) = linear col*128 + p so chunks of columns are
        # contiguous, sign-pure ranges of the bucket
        idx2d = np.ascontiguousarray(idx_lin.reshape(COLS, P).T)

        shard = np.empty((TOTAL + 1, 1), np.float32)
        shard[:TOTAL, 0] = pv_flat[c * TOTAL : (c + 1) * TOTAL]
        shard[TOTAL, 0] = 0.0
        in_maps.append({"pv": shard, "idx": idx2d})

    if _NC is None:
        _NC = _build_nc()

    res = run_bass_kernel_spmd(
        _NC, in_maps, core_ids=list(range(NCORES)), trace=TRACE
    )
    LAST_RESULTS = res

    # device sums sig(+s) over positives, sig(-s) over negatives, 0.5/pad;
    # sum w*sig(s) = dev_sum - n_neg_dev - 0.5*pads
    asig = host_extra - correction
    for c in range(NCORES):
        asig += float(np.asarray(res.results[c]["out"], dtype=np.float64).sum())

    neg = float(np.sum(lab == 0))
    loss = -(neg + asig) / ((1.0 + neg) * float(N))
    return np.array([loss], dtype=np.float32)
